# revision 93
# baseline (speedup 1.0000x reference)
"""GQA attention kernel for Trainium2, sharded over 8 NeuronCores.

Problem: B=2, S=2048, D=2048, H=16 query heads, KV=4 kv heads, HD=128,
RoPE, no causal mask, out = softmax(q k^T / sqrt(HD)) v @ Wo.

Sharding: core = b*4 + g  (b in {0,1} batch, g in {0..3} head group).
Each core handles 4 query heads [4g..4g+3] and kv head g (exact GQA
split), with Wq/Wk/Wv column-sliced and Wo row-sliced.  Each core
produces a partial o_proj output for its batch; host sums the 4 partials
per batch.

Per-core layout strategy:
  - host supplies h^T pre-tiled AND split into fp8e4 value+residual
    (pre-scaled by 8 out of the e4m3 subnormal floor); Wq/Wk/Wv the
    same (pre-scaled by 512, descale folded into the RoPE tables)
  - QKV projections run as residual-compensated fp8 DoubleRow:
    h@W = h8@W8 + dh8@W8 + h8@dW8, 2 ko-chunks per matmul at 0.5
    cycles/row -> 25% fewer PE cycles than bf16 at ~4x less error
  - RoPE applied in fp32 with head-broadcast APs (6 wide DVE ops);
    q_rot/k_rot PE-transposed to qT/kT [HD, S].  kT (which gates all
    of attention) is computed for every tile in phase 1, but the last
    4 tiles' ENTIRE q projections defer into block 0's head-blocks as
    exp-independent PE filler: 4 short-held 128-column DoubleRow
    chunk accumulations each, copies on ACT, RoPE on DVE, transposes
    trailing one head-block so no engine latency parks the PE
  - scores^T tiles = matmul(lhsT=kT_tile, rhs=qT_block), bf16, in
    PAIRS into one 2-bank [128,1024] PSUM tile so a single wide exp on
    ScalarE covers both; emission is software-pipelined (scores of
    pair j+1 before PV of pair j) so PE never waits on exp
  - exp straight out of PSUM, no max subtraction (scores ~N(0,1))
  - out^T = sum_c matmul(lhsT=v_chunk, rhs=expT_half), bf16
  - softmax denominators via N=1 matmuls (lhsT=expT 128-q slice,
    rhs=ones[128,1]; the ones hold 16 to rescale the fp8 attention
    out) -> essentially free on the PE; partials are accumulated into
    SBUF by the DVE, reciprocal'd, PE-transposed to a [1,512] row and
    partition-broadcast by the (otherwise idle) GpSimd engine; the
    normalized attn-out is then split into fp8 value + residual
    (3 DVE ops) for the o_proj
  - o_proj: residual-compensated fp8 DoubleRow like the projections
    (ao8@wo8 + dao8@wo8 + ao8@dwo8, Wo host-split and pre-scaled by
    512; the net 256*512 output scale divided out on the host);
    emitted interleaved (4 units of [128,512] per attention head-block,
    one block behind) so o_proj fills the PE gaps while ScalarE works
    through the exps; tail units run in nblk pairs through the wide
    PSUM slots with DVE/ACT-alternating copies; y DMA'd as fp16
  - per head-block, the finish-chain (bf16 reciprocal transposes +
    broadcast + normalize-and-split) is deferred past the next
    head-block's first score pair so PE never parks on DVE latency
  - PSUM (8 banks): wide pool 2x2-bank slots (score pairs / ph1 ps_q /
    tail y pairs), B pool 2x1 (ph1 transposes / ph2 sums scratch,
    recip rows + o_proj ps_y), C pool 2x1 (ph1 kv / ph2 PV accum)

Cost-model timeline (TimelineSim): ~264 us/core (baseline 330), PE
~83% busy (220 us), ACT ~155 us (exp-gated attention inner loop).
NOTE: GpSimd must never touch PSUM — CoreSim/TimelineSim accept it
but the real NEFF/BIR verifier rejects it.
"""

import math
import numpy as np
import ml_dtypes

B, S, D = 2, 2048, 2048
H, KV, HD = 16, 4, 128
G = 4          # tensor-parallel head groups
HG = H // G    # 4 query heads per core
QCOLS = HG * HD  # 512
P = 128
NT = S // P    # 16 sequence tiles
KO = D // P    # 16 contraction chunks
NB = S // 512  # 4 query blocks of 512

BF16 = ml_dtypes.bfloat16
F8 = ml_dtypes.float8_e4m3

# device y is scaled by (HS*WS/16) * WSO = 256 * 512 (see make_in_maps)
Y_DESCALE = 1.0 / (256.0 * 512.0)

_CACHE = {}


def _split8(x):
    """fp32 -> (fp8 value, fp8 residual): x ~= x8 + dx8 to ~0.1%."""
    x8 = x.astype(F8)
    dx8 = (x - x8.astype(np.float32)).astype(F8)
    return np.stack([x8, dx8])


def _build_nc():
    import concourse.mybir as mybir
    import concourse.tile as tile
    from concourse import bacc
    from concourse.masks import make_identity
    from contextlib import ExitStack

    dt = mybir.dt
    nc = bacc.Bacc(
        "TRN2",
        target_bir_lowering=False,
        debug=False,
        enable_asserts=False,
        num_devices=8,
    )

    # h^T pre-tiled on host and split into fp8e4 value + fp8e4 residual
    # (h = h8 + dh8 to ~0.1%): hT8[i, t, p, ko, sc] with t=0 the value and
    # t=1 the residual; each DMA'd s-tile is contiguous per partition
    f8 = dt.float8e4
    hT = nc.dram_tensor(
        "hT", [S // 128, 2, 128, (D // 128) * 128], f8, kind="ExternalInput"
    ).ap()
    wq = nc.dram_tensor("wq", [2, D, QCOLS], f8, kind="ExternalInput").ap()
    wk = nc.dram_tensor("wk", [2, D, HD], f8, kind="ExternalInput").ap()
    wv = nc.dram_tensor("wv", [2, D, HD], f8, kind="ExternalInput").ap()
    wo = nc.dram_tensor("wo", [2, QCOLS, D], f8, kind="ExternalInput").ap()
    cosd = nc.dram_tensor("cosd", [S, HD], dt.float32, kind="ExternalInput").ap()
    sind = nc.dram_tensor("sind", [S, HD], dt.float32, kind="ExternalInput").ap()
    # fp16 partials: halves the output DMA; the host accumulates in fp32.
    # fp16 (not bf16) keeps the partial quantization at ~0.05%
    y = nc.dram_tensor("y", [S, D], dt.float16, kind="ExternalOutput").ap()

    with tile.TileContext(nc) as tc:
        _emit(tc, nc, mybir, hT, wq, wk, wv, wo, cosd, sind, y, make_identity)

    nc.compile()
    return nc


def _emit(tc, nc, mybir, hT, wq, wk, wv, wo, cosd, sind, y, make_identity):
    import os
    from contextlib import ExitStack

    PHASES = os.environ.get("K_PHASES", "123")

    dt = mybir.dt
    bf16 = dt.bfloat16
    f32 = dt.float32
    f8 = dt.float8e4
    DR = mybir.MatmulPerfMode.DoubleRow
    Exp = mybir.ActivationFunctionType.Exp

    with ExitStack() as ctx:
        const = ctx.enter_context(tc.tile_pool(name="const", bufs=1))
        wpool = ctx.enter_context(tc.tile_pool(name="wpool", bufs=1))
        big = ctx.enter_context(tc.tile_pool(name="big", bufs=1))
        hpool = ctx.enter_context(tc.tile_pool(name="hpool", bufs=8))
        work = ctx.enter_context(tc.tile_pool(name="work", bufs=4))
        expp = ctx.enter_context(tc.tile_pool(name="expp", bufs=6))
        # PSUM: "wide" = 2-bank slots for paired score tiles (also holds
        # phase-1 ps_q); B = ph1 transposes / ph2 recip rows + o_proj ps_y;
        # C = ph1 kv / ph2 PV accumulators.  2*2 + 2 + 2 = 8 banks.
        ps_wide = ctx.enter_context(tc.tile_pool(name="ps_wide", bufs=2, space="PSUM"))
        ps_b = ctx.enter_context(tc.tile_pool(name="ps_b", bufs=2, space="PSUM"))
        ps_c = ctx.enter_context(tc.tile_pool(name="ps_c", bufs=2, space="PSUM"))

        # --- constants ---
        ident = const.tile([P, P], bf16)
        make_identity(nc, ident)
        # 16 (not 1) so the reciprocal carries a 1/16 rescale that pulls
        # the fp8 attention-out values into e4m3's comfortable range
        ones1 = const.tile([P, 1], bf16)
        nc.vector.memset(ones1, 16.0)

        # --- hT prefetch helper (pre-tiled on host: hT[i] = [128, KO*128]) --
        ht_tiles = {}

        def load_ht(i):
            if i not in ht_tiles:
                hT_t = hpool.tile([P, 2, KO, P], f8, tag="ht", name=f"ht{i}")
                for t in range(2):
                    nc.sync.dma_start(
                        hT_t[:, t], hT[i, t].rearrange("p (ko s) -> p ko s", ko=KO)
                    )
                ht_tiles[i] = hT_t
            return ht_tiles[i]

        # --- weights and tables to SBUF ---
        # DMA emission order drives the model's serial DMA queue: first two
        # hT tiles and the first weight chunks go first so the projection
        # matmuls can start immediately; wo (phase 3) goes last.
        wq_sb = wpool.tile([P, 2, KO, QCOLS], f8)
        wkv_sb = wpool.tile([P, 2, KO, 2 * HD], f8)
        cos_sb = wpool.tile([P, NT, HD], f32)
        sin_sb = wpool.tile([P, NT, HD], f32)
        wq_r = wq.rearrange("t (ko p) m -> t p ko m", p=P)
        wk_r = wk.rearrange("t (ko p) m -> t p ko m", p=P)
        wv_r = wv.rearrange("t (ko p) m -> t p ko m", p=P)
        cos_r = cosd.rearrange("(i p) c -> p i c", p=P)
        sin_r = sind.rearrange("(i p) c -> p i c", p=P)
        # startup-critical order: the fp8 VALUE streams (term 0 of the
        # first s-tile) go first, then the residual streams, matching the
        # term-outer projection loop; h tiles interleave between groups
        KG = 4
        if "1" in PHASES:
            load_ht(0)
        for kg in range(0, KO, KG):
            ks = slice(kg, kg + KG)
            nc.sync.dma_start(wq_sb[:, 0, ks], wq_r[0, :, ks])
            nc.sync.dma_start(wkv_sb[:, 0, ks, :HD], wk_r[0, :, ks])
            nc.sync.dma_start(wkv_sb[:, 0, ks, HD:], wv_r[0, :, ks])
            if "1" in PHASES and kg == 0:
                load_ht(1)
        for kg in range(0, KO, KG):
            ks = slice(kg, kg + KG)
            nc.sync.dma_start(wq_sb[:, 1, ks], wq_r[1, :, ks])
            nc.sync.dma_start(wkv_sb[:, 1, ks, :HD], wk_r[1, :, ks])
            nc.sync.dma_start(wkv_sb[:, 1, ks, HD:], wv_r[1, :, ks])
        if "1" in PHASES:
            load_ht(2)
            load_ht(3)
        for kg in range(0, KO, KG):
            ts_ = slice(kg, kg + KG)  # 4 s-tiles of rope tables per chunk
            nc.sync.dma_start(cos_sb[:, ts_], cos_r[:, ts_])
            nc.sync.dma_start(sin_sb[:, ts_], sin_r[:, ts_])

        # --- persistent intermediates ---
        # qT and kT fused: [hd, 5, s] with slots 0..3 = q heads, slot 4 = k
        qkT = big.tile([P, HG + 1, S], bf16)
        qT = qkT[:, :HG]                   # [hd, head, s]
        kT = qkT[:, HG]                    # [hd, s]
        v_sb = big.tile([P, NT, HD], bf16)  # [s_inner, s_chunk, hd]
        # attn_out^T [c_inner, head, s] as fp8 value + residual so o_proj
        # can run as residual-compensated DoubleRow like the projections
        ao8T = big.tile([P, HG, S], f8)
        dao8T = big.tile([P, HG, S], f8)

        # ---------------- Phase 1: QKV projections + RoPE + transposes ------
        PROJ_TERMS = [(0, 0), (1, 0), (0, 1)]
        late_qrot = []
        for i in range(NT if "1" in PHASES else 0):
            hT_t = load_ht(i)
            if i + 2 < NT:
                load_ht(i + 2)

            has_q = i < 12
            if has_q:
                ps_q = ps_wide.tile(
                    [P, 1024], f32, tag="wide", name="ps_q"
                )[:, :512]
            ps_kv = ps_c.tile([P, 512], f32, tag="c", name="ps_kv")[:, : 2 * HD]
            # residual-compensated fp8 projection: h@W = h8@W8 + dh8@W8
            # + h8@dW8 (error ~0.1%, better than bf16), each term running
            # as DoubleRow over ko-chunk pairs at 0.5 cycles/row.  Term
            # outer so the first pass only needs the fp8 value streams
            for t, (ht_i, w_i) in enumerate(PROJ_TERMS):
                for jp in range(KO // 2):
                    ks = slice(2 * jp, 2 * jp + 2)
                    first = t == 0 and jp == 0
                    last = t == 2 and jp == KO // 2 - 1
                    if has_q:
                        nc.tensor.matmul(
                            ps_q, hT_t[:, ht_i, ks], wq_sb[:, w_i, ks],
                            start=first, stop=last, perf_mode=DR,
                        )
                    nc.tensor.matmul(
                        ps_kv, hT_t[:, ht_i, ks], wkv_sb[:, w_i, ks],
                        start=first, stop=last, perf_mode=DR,
                    )

            # v: straight cast copy into [s, hd] layout; route the last
            # tiles' copies to DVE so ACT is free when attention starts
            cp = nc.vector if i >= NT - 3 else nc.scalar
            if cp is nc.vector:
                nc.vector.tensor_copy(v_sb[:, i], ps_kv[:, HD:])
            else:
                nc.scalar.copy(v_sb[:, i], ps_kv[:, HD:])

            # q and k side by side in one [P, 5, HD] fp32 tile for fused RoPE
            qk_f = work.tile([P, HG + 1, HD], f32, tag="qkf")
            if has_q:
                if cp is nc.vector:
                    nc.vector.tensor_copy(
                        qk_f[:, :HG], ps_q.rearrange("p (h c) -> p h c", h=HG)
                    )
                else:
                    nc.scalar.copy(
                        qk_f[:, :HG], ps_q.rearrange("p (h c) -> p h c", h=HG)
                    )
            if cp is nc.vector:
                nc.vector.tensor_copy(qk_f[:, HG], ps_kv[:, :HD])
            else:
                nc.scalar.copy(qk_f[:, HG], ps_kv[:, :HD])

            HF = HD // 2

            def do_rope(src, lo_h, n_h, i=i):
                # returns bf16 RoPE(src[:, lo_h:lo_h+n_h]) as [P, n_h, HD]
                cos_t = cos_sb[:, i]
                sin_t = sin_sb[:, i]
                cos_lo = cos_t[:, None, :HF].to_broadcast((P, n_h, HF))
                cos_hi = cos_t[:, None, HF:].to_broadcast((P, n_h, HF))
                sin_lo = sin_t[:, None, :HF].to_broadcast((P, n_h, HF))
                sin_hi = sin_t[:, None, HF:].to_broadcast((P, n_h, HF))
                s = src[:, lo_h : lo_h + n_h]
                s_lo = s[:, :, :HF]
                s_hi = s[:, :, HF:]
                rot = work.tile(
                    [P, HG + 1, HD], bf16, tag="qkrot", name="rot"
                )[:, :n_h]
                t1 = work.tile([P, HG + 1, HF], f32, tag="rt1", name="t1")[:, :n_h]
                t2 = work.tile([P, HG + 1, HF], f32, tag="rt2", name="t2")[:, :n_h]
                nc.vector.tensor_mul(t1, s_lo, cos_lo)
                nc.vector.tensor_mul(t2, s_hi, sin_lo)
                nc.vector.tensor_sub(rot[:, :, :HF], t1, t2)
                t3 = work.tile([P, HG + 1, HF], f32, tag="rt1", name="t3")[:, :n_h]
                t4 = work.tile([P, HG + 1, HF], f32, tag="rt2", name="t4")[:, :n_h]
                nc.vector.tensor_mul(t3, s_hi, cos_hi)
                nc.vector.tensor_mul(t4, s_lo, sin_hi)
                nc.vector.tensor_add(rot[:, :, HF:], t3, t4)
                return rot

            if i < 12:
                # fused RoPE over q heads + k, then all 5 transposes
                qk_rot = do_rope(qk_f, 0, HG + 1)
                ps_tk = ps_b.tile([P, P], bf16, tag="b", name="ps_tk")
                nc.tensor.transpose(ps_tk, qk_rot[:, HG], ident)
                nc.vector.tensor_copy(kT[:, i * P : (i + 1) * P], ps_tk)
                ps_t = ps_b.tile([P, HG * P], bf16, tag="b", name="ps_t")
                for h in range(HG):
                    nc.tensor.transpose(
                        ps_t[:, h * P : (h + 1) * P], qk_rot[:, h], ident
                    )
                nc.vector.tensor_copy(
                    qT[:, :, i * P : (i + 1) * P],
                    ps_t.rearrange("p (h s) -> p h s", h=HG),
                )
            else:
                # tiles 8..15: k-only RoPE now (kT gates ALL of phase 2);
                # the full q projection+RoPE is deferred into head-blocks
                # 0..7 as exp-independent PE filler
                k_rot = do_rope(qk_f, HG, 1)
                ps_tk = ps_b.tile([P, P], bf16, tag="b", name="ps_tk")
                nc.tensor.transpose(ps_tk, k_rot[:, 0], ident)
                nc.vector.tensor_copy(kT[:, i * P : (i + 1) * P], ps_tk)

        # wo is only needed for o_proj: load it while phase 2 runs
        wo_sb = wpool.tile([P, 2, HG, D], f8)
        for t in range(2):
            nc.sync.dma_start(
                wo_sb[:, t], wo[t].rearrange("(ch p) n -> p ch n", p=P)
            )

        # ------- Phase 2 (attention) with o_proj units interleaved ----------
        y_r = y.rearrange("(i p) n -> p i n", p=P)
        pend_oproj = []

        def emit_oproj_unit(i, nblk, cp=None, wide=False):
            if wide:
                ps_y = ps_wide.tile([P, 1024], f32, tag="wide", name="ps_yw")[:, :512]
            else:
                ps_y = ps_b.tile([P, 512], f32, tag="b", name="ps_y")
            ns = slice(nblk * 512, (nblk + 1) * 512)
            isl = slice(i * P, (i + 1) * P)
            oterms = [(ao8T, 0), (dao8T, 0), (ao8T, 1)]
            for t, (ao, w_i) in enumerate(oterms):
                for hh in range(HG // 2):
                    cs = slice(2 * hh, 2 * hh + 2)
                    nc.tensor.matmul(
                        ps_y,
                        ao[:, cs, isl],
                        wo_sb[:, w_i, cs, ns],
                        start=(t == 0 and hh == 0),
                        stop=(t == len(oterms) - 1 and hh == HG // 2 - 1),
                        perf_mode=DR,
                    )
            y_sb = work.tile([P, 512], dt.float16, tag="ysb", bufs=4)
            if cp is nc.scalar:
                nc.scalar.copy(y_sb, ps_y)
            else:
                nc.vector.tensor_copy(y_sb, ps_y)
            nc.sync.dma_start(y_r[:, i, ns], y_sb)

        def drain_oproj(n, cp=None, wide=False):
            for _ in range(min(n, len(pend_oproj))):
                emit_oproj_unit(*pend_oproj.pop(0), cp=cp, wide=wide)

        # finish-chain of the previous head-block, deferred into the current
        # one so the PE never waits on the DVE reciprocal latency
        prev_finish = [None]

        HF2 = HD // 2

        def rope_deferred(qk_f, i):
            # q RoPE for a deferred tile (same math as phase 1's do_rope)
            cos_t = cos_sb[:, i]
            sin_t = sin_sb[:, i]
            cos_lo = cos_t[:, None, :HF2].to_broadcast((P, HG, HF2))
            cos_hi = cos_t[:, None, HF2:].to_broadcast((P, HG, HF2))
            sin_lo = sin_t[:, None, :HF2].to_broadcast((P, HG, HF2))
            sin_hi = sin_t[:, None, HF2:].to_broadcast((P, HG, HF2))
            s = qk_f[:, :HG]
            s_lo = s[:, :, :HF2]
            s_hi = s[:, :, HF2:]
            rot = work.tile(
                [P, HG + 1, HD], bf16, tag="qkrot", name="rotd"
            )[:, :HG]
            t1 = work.tile([P, HG + 1, HF2], f32, tag="rt1", name="td1")[:, :HG]
            t2 = work.tile([P, HG + 1, HF2], f32, tag="rt2", name="td2")[:, :HG]
            nc.vector.tensor_mul(t1, s_lo, cos_lo)
            nc.vector.tensor_mul(t2, s_hi, sin_lo)
            nc.vector.tensor_sub(rot[:, :, :HF2], t1, t2)
            t3 = work.tile([P, HG + 1, HF2], f32, tag="rt1", name="td3")[:, :HG]
            t4 = work.tile([P, HG + 1, HF2], f32, tag="rt2", name="td4")[:, :HG]
            nc.vector.tensor_mul(t3, s_hi, cos_hi)
            nc.vector.tensor_mul(t4, s_lo, sin_hi)
            nc.vector.tensor_add(rot[:, :, HF2:], t3, t4)
            return rot

        def emit_qchunk(i, c, qk_f):
            # one 128-column chunk of a deferred q projection: 24 short
            # DoubleRow matmuls through a briefly-held B-pool bank; the
            # PSUM->SBUF copy rides the DVE (GpSimd cannot touch PSUM)
            ps_qc = ps_b.tile([P, P], f32, tag="b", name="ps_qc")
            for t, (ht_i, w_i) in enumerate(PROJ_TERMS):
                for jp in range(KO // 2):
                    ks = slice(2 * jp, 2 * jp + 2)
                    nc.tensor.matmul(
                        ps_qc,
                        ht_tiles[i][:, ht_i, ks],
                        wq_sb[:, w_i, ks, c * P : (c + 1) * P],
                        start=(t == 0 and jp == 0),
                        stop=(t == 2 and jp == KO // 2 - 1),
                        perf_mode=DR,
                    )
            nc.scalar.copy(qk_f[:, c], ps_qc)

        # deferred-tile state: tiles 8..15 processed one per head-block
        # (hbs 0..7); the transposes trail one head-block behind the rope
        deferred_q = list(range(12, NT)) if "1" in PHASES else []
        prev_qfin = [None]

        for b in range(NB if "2" in PHASES else 0):
            qs = slice(b * 512, (b + 1) * 512)
            for h in range(HG):
                ps_o = ps_c.tile([P, 512], f32, tag="c", name="ps_o")
                sums8 = work.tile([P, 8], f32, tag="sums8")
                d_i = deferred_q.pop(0) if deferred_q else None
                if d_i is not None:
                    d_qkf = work.tile([P, HG + 1, HD], f32, tag="qkf")

                # software-pipelined: scores/exp for pair j+1 are emitted
                # before PV of pair j so PE never waits on the exp
                def emit_scores(j):
                    ps_s2 = ps_wide.tile(
                        [P, 1024], f32, tag="wide", name="ps_s2"
                    )
                    for r in range(2):
                        c = 2 * j + r
                        nc.tensor.matmul(
                            ps_s2[:, r * 512 : (r + 1) * 512],
                            kT[:, c * P : (c + 1) * P],
                            qT[:, h, qs],
                            start=True,
                            stop=True,
                        )
                    expT = expp.tile([P, 1024], bf16, tag="exp", name="expT")
                    nc.scalar.activation(expT, ps_s2, Exp)
                    return ps_s2, expT

                def emit_pv(j, e):
                    first, last = j == 0, j == NT // 2 - 1
                    for r in range(2):
                        c = 2 * j + r
                        sl = slice(r * 512, (r + 1) * 512)
                        nc.tensor.matmul(
                            ps_o, v_sb[:, c], e[:, sl],
                            start=(first and r == 0), stop=(last and r == 1),
                        )

                def emit_sums(j, e):
                    # 8 denominator partials as N=1 matmuls (lhsT = expT
                    # 128-q slice, rhs = ones) into a small scratch PSUM
                    # tile, then one DVE accumulate into SBUF.  Kept off the
                    # score-slot recycle path so exp cadence is unaffected.
                    ps_sp = ps_b.tile([P, 8], f32, tag="b", name="ps_sp")
                    for r in range(2):
                        for js in range(4):
                            nc.tensor.matmul(
                                ps_sp[:, r * 4 + js : r * 4 + js + 1],
                                e[:, r * 512 + js * P : r * 512 + (js + 1) * P],
                                ones1,
                                start=True,
                                stop=True,
                                skip_group_check=True,
                            )
                    if j == 0:
                        nc.vector.tensor_copy(sums8, ps_sp)
                    else:
                        nc.vector.tensor_add(sums8, sums8, ps_sp)

                # o_proj units slot in at odd pairs, between the scores and
                # the exp-gated PV, so the in-order PE stream has cover while
                # ScalarE works.  The deferred finish-chain of the previous
                # head-block goes at j==1; at a block boundary (h==0) it must
                # precede the drained unit (the unit reads the aoT row that
                # finish writes), elsewhere the order favors the unit first.
                def fin():
                    if prev_finish[0] is not None:
                        prev_finish[0]()
                        prev_finish[0] = None

                pairs = []
                for j in range(NT // 2):
                    pairs.append(emit_scores(j))
                    if j == 1 and h == 0:
                        fin()
                    if j % 2 == 1:
                        drain_oproj(1)
                    if j == 1 and h != 0:
                        fin()
                    if j == 2 and prev_qfin[0] is not None:
                        prev_qfin[0]()
                        prev_qfin[0] = None
                    if j in (2, 4, 6) and d_i is not None:
                        emit_qchunk(d_i, j // 2 - 1, d_qkf)
                    if j >= 1:
                        s2p, ep = pairs[j - 1]
                        emit_pv(j - 1, ep)
                        emit_sums(j - 1, ep)
                s2p, ep = pairs[-1]
                emit_pv(NT // 2 - 1, ep)
                emit_sums(NT // 2 - 1, ep)

                # fold 8 partials -> 4 q-slice sums -> reciprocals (DVE);
                # the PE part of the finish is deferred into the next block
                sums4 = work.tile([P, 4], f32, tag="sums4")
                nc.vector.tensor_add(sums4, sums8[:, 0:4], sums8[:, 4:8])
                # bf16 reciprocals: halves the transpose cost on the PE
                # (bf16 transposes run 1 cycle/row vs fp32's 2); the 0.4%
                # denominator quantization is common-mode and stays well
                # inside the error budget
                recip4 = work.tile([P, 4], bf16, tag="recip4")
                with nc.allow_low_precision(
                    reason="bf16 softmax reciprocals: 0.4% common-mode, "
                    "5x inside the correctness budget"
                ):
                    nc.vector.reciprocal(recip4, sums4)

                if d_i is not None:
                    emit_qchunk(d_i, 3, d_qkf)
                    q_rot_d = rope_deferred(d_qkf, d_i)

                    def qfin(i=d_i, q_rot_d=q_rot_d):
                        ps_t = ps_b.tile(
                            [P, HG * P], bf16, tag="b", name="ps_td"
                        )
                        for hh in range(HG):
                            nc.tensor.transpose(
                                ps_t[:, hh * P : (hh + 1) * P],
                                q_rot_d[:, hh],
                                ident,
                            )
                        nc.vector.tensor_copy(
                            qT[:, :, i * P : (i + 1) * P],
                            ps_t.rearrange("p (h s) -> p h s", h=HG),
                        )

                    prev_qfin[0] = qfin

                def finish(ps_o=ps_o, recip4=recip4, h=h, qs=qs):
                    # transpose recips [q,4] -> [1,512] row, broadcast to
                    # all partitions on GpSimd, normalize out of PSUM
                    ps_row = ps_b.tile([P, 512], bf16, tag="b", name="ps_row")
                    for js in range(4):
                        nc.tensor.transpose(
                            ps_row[0:1, js * P : (js + 1) * P],
                            recip4[:, js : js + 1],
                            ident,
                        )
                    row_sb = work.tile([P, 512], bf16, tag="row")
                    nc.vector.tensor_copy(row_sb[0:1, :], ps_row[0:1, :])
                    recip_bc = work.tile([P, 512], bf16, tag="rbc")
                    nc.gpsimd.partition_broadcast(recip_bc, row_sb[0:1, :])
                    ao_f = work.tile([P, 512], f32, tag="aof")
                    nc.vector.tensor_mul(ao_f, ps_o, recip_bc)
                    nc.vector.tensor_copy(ao8T[:, h, qs], ao_f)
                    nc.vector.tensor_sub(dao8T[:, h, qs], ao_f, ao8T[:, h, qs])

                prev_finish[0] = finish
            pend_oproj += [
                (i, nblk) for i in range(4 * b, 4 * b + 4) for nblk in range(4)
            ]

        # ---------------- Phase 3: o_proj tail ------------------------------
        if "2" in PHASES and prev_qfin[0] is not None:
            prev_qfin[0]()
            prev_qfin[0] = None
        if "2" in PHASES and prev_finish[0] is not None:
            prev_finish[0]()
            prev_finish[0] = None
        if "3" in PHASES:
            # tail drain: ScalarE and the wide PSUM pool are idle now.
            # Units run in nblk pairs through the two wide slots so each
            # pair costs one [128,1024] copy + one DMA, with the copies
            # alternating between DVE and ACT
            k = 0
            while pend_oproj:
                i, n0 = pend_oproj.pop(0)
                _, n1 = pend_oproj.pop(0)
                ps_yw = ps_wide.tile([P, 1024], f32, tag="wide", name="ps_yw")
                for half, nblk in enumerate((n0, n1)):
                    ns = slice(nblk * 512, (nblk + 1) * 512)
                    isl = slice(i * P, (i + 1) * P)
                    psy = ps_yw[:, half * 512 : (half + 1) * 512]
                    oterms = [(ao8T, 0), (dao8T, 0), (ao8T, 1)]
                    # first tail pair: heads (0,1) lead across all terms so
                    # the PE has cover while the last head-block's finish
                    # chain (which writes heads 2-3's ao rows) drains
                    if k == 0:
                        order = [(t, hh) for hh in range(HG // 2) for t in range(3)]
                    else:
                        order = [(t, hh) for t in range(3) for hh in range(HG // 2)]
                    for n_o, (t, hh) in enumerate(order):
                        ao, w_i = oterms[t]
                        cs = slice(2 * hh, 2 * hh + 2)
                        nc.tensor.matmul(
                            psy,
                            ao[:, cs, isl],
                            wo_sb[:, w_i, cs, ns],
                            start=(n_o == 0),
                            stop=(n_o == len(order) - 1),
                            perf_mode=DR,
                        )
                y_sb = work.tile([P, 1024], dt.float16, tag="ysb2", bufs=3)
                if k % 2:
                    nc.scalar.copy(y_sb, ps_yw)
                else:
                    nc.vector.tensor_copy(y_sb, ps_yw)
                nc.sync.dma_start(y_r[:, i, n0 * 512 : (n0 + 2) * 512], y_sb)
                k += 1


def get_nc():
    if "nc" not in _CACHE:
        _CACHE["nc"] = _build_nc()
    return _CACHE["nc"]


def make_in_maps(inputs):
    """Shard full inputs into 8 per-core input maps."""
    # HS/WS lift h and the projection weights out of fp8e4's subnormal
    # range (weights have std ~0.02, right at the 2^-9 floor) so the
    # fp8 value+residual split keeps ~0.1% precision.  The inverse
    # 2^-12 is folded into the RoPE tables (descales q AND k exactly)
    # and into Wo (descales v through the attention output).
    HS, WS = 8.0, 512.0
    h = np.asarray(inputs["hidden_states"], dtype=np.float32) * HS
    cos = np.asarray(inputs["cos"], dtype=np.float32).reshape(S, HD) / (HS * WS)
    sin = np.asarray(inputs["sin"], dtype=np.float32).reshape(S, HD) / (HS * WS)
    # fold the 1/sqrt(HD) softmax scale into Wq before the fp8 split
    Wq = np.asarray(inputs["Wq"], dtype=np.float32) * (WS * HD ** -0.5)
    Wk = np.asarray(inputs["Wk"], dtype=np.float32) * WS
    Wv = np.asarray(inputs["Wv"], dtype=np.float32) * WS
    # the v-path scale HS*WS is undone via the ones column (16) and the
    # final host-side Y_DESCALE, so Wo only carries the WSO lift below
    Wo = np.asarray(inputs["Wo"], dtype=np.float32)

    # hT4[i, t, p, ko*128+sc] = split8(h[b].T)[t][ko*128+p, i*128+sc]
    hT = [
        _split8(
            np.ascontiguousarray(
                h[b].T.reshape(KO, P, NT, P).transpose(2, 1, 0, 3).reshape(NT, P, KO * P)
            )
        ).transpose(1, 0, 2, 3).copy()
        for b in range(B)
    ]
    wq_s = [_split8(np.ascontiguousarray(Wq[:, g * QCOLS : (g + 1) * QCOLS])) for g in range(G)]
    wk_s = [_split8(np.ascontiguousarray(Wk[:, g * HD : (g + 1) * HD])) for g in range(G)]
    wv_s = [_split8(np.ascontiguousarray(Wv[:, g * HD : (g + 1) * HD])) for g in range(G)]
    # WSO lifts Wo out of the fp8 subnormal floor; the attention-out side
    # already carries HS*WS/16 = 256, so the device y is scaled by
    # 256*WSO and kernel() divides it back out after the gather
    WSO = 512.0
    wo_s = [
        _split8(np.ascontiguousarray(Wo[g * QCOLS : (g + 1) * QCOLS, :]) * WSO)
        for g in range(G)
    ]

    in_maps = []
    for core in range(8):
        b, g = divmod(core, G)
        in_maps.append(
            {
                "hT": hT[b],
                "wq": wq_s[g],
                "wk": wk_s[g],
                "wv": wv_s[g],
                "wo": wo_s[g],
                "cosd": cos,
                "sind": sin,
            }
        )
    return in_maps


def kernel(**inputs) -> np.ndarray:
    from concourse import bass_utils

    nc = get_nc()
    in_maps = make_in_maps(inputs)
    res = bass_utils.run_bass_kernel_spmd(nc, in_maps, core_ids=list(range(8)))
    out = np.zeros((B, S, D), dtype=np.float32)
    for core in range(8):
        b = core // G
        out[b] += res.results[core]["y"]
    # undo the device-side output scaling (see make_in_maps: ao carries
    # HS*WS/16 = 256, Wo carries WSO = 512)
    out *= Y_DESCALE
    return out
